# revision 1
# baseline (speedup 1.0000x reference)
"""Trainium2 Bass kernel: BinarizeLinear inference.

Computes out = sign01(x) @ weight + bias where sign01(t) = +1 if t > 0 else -1,
for x [8192, 4096] f32, weight [4096, 4096] f32, bias [4096] f32.

Strategy: data-parallel over the token dim across 8 NeuronCores (each core
gets x[c*1024:(c+1)*1024], the full weight, and the bias). No collectives;
outputs are concatenated on the host.

Per-core kernel:
  - binarize x tiles to bf16 {+1,-1} on the Scalar engine (Sign activation,
    with a tiny negative bias so exact zeros map to -1 like the reference),
  - transpose the binarized activations 128x128 via the PE transpose path so
    the contraction dim lands on partitions (kept resident in SBUF, 8 MB),
  - stream weight in 512-column chunks, cast f32->bf16 in-flight via SWDGE
    cast-DMA (xb is exactly representable in bf16, so a single bf16 pass
    only rounds the weight: measured rel L2 err ~1.7e-3; an optional
    hi/lo split-weight mode reaches ~2.5e-6 at 2x the matmul work),
  - accumulate out tiles [128m, 512n] in PSUM over 32 k-tiles,
  - evict with a fused bias add on the Vector engine, DMA to DRAM.
"""

import contextlib
import os
import sys

import numpy as np

os.environ.setdefault("JAX_PLATFORMS", "axon")

for _p in ("/opt/trn_rl_repo", "/root/.axon_site/_ro/trn_rl_repo"):
    if os.path.isdir(_p) and _p not in sys.path:
        sys.path.insert(0, _p)
        break

import concourse.bass as bass  # noqa: E402
import concourse.mybir as mybir  # noqa: E402
import concourse.tile as tile  # noqa: E402
from concourse import bacc  # noqa: E402
from concourse.bass_utils import run_bass_kernel_spmd  # noqa: E402
from concourse.masks import make_identity  # noqa: E402

P = 128
N_CORES = 8
TOKENS, IN_F, OUT_F = 8192, 4096, 4096
F32 = mybir.dt.float32
BF16 = mybir.dt.bfloat16

# hi/lo split-weight mode: 2x matmul passes, ~fp32 accuracy. Off by default.
HILO = bool(int(os.environ.get("BINLIN_HILO", "0")))
# transpose path for the binarized activations: "host" (x arrives
# pre-transposed, no on-device transpose), "pe" (tensor engine) or
# "dma" (xbar DMA transpose, keeps the tensor engine free)
XPOSE = os.environ.get("BINLIN_XPOSE", "host")
# weight f32->bf16 path: "swdge" (cast during DMA) or "act" (HWDGE f32 DMA
# + scalar-engine convert)
WLOAD = os.environ.get("BINLIN_WLOAD", "swdge")


def build_nc(
    m_shard=TOKENS // N_CORES,
    k=IN_F,
    n=OUT_F,
    n_chunk=512,
    hilo=HILO,
    loop_k=1,
    xpose=None,
    wload=None,
    wbufs=2,
    loop_scope="all",
):
    """loop_k > 1 wraps the whole body in a hardware For loop that repeats
    the identical computation; used only for wall-clock slope timing."""
    mt_n = m_shard // P
    kt_n = k // P
    nt_n = n // n_chunk
    assert m_shard % P == 0 and k % P == 0 and n % n_chunk == 0

    nc = bacc.Bacc(
        "TRN2", target_bir_lowering=False, debug=False, num_devices=N_CORES
    )
    # in "host" xpose mode the x shard arrives pre-transposed as [k, m_shard]
    x_shape = [k, m_shard] if (xpose or XPOSE) == "host" else [m_shard, k]
    x_ap = nc.declare_dram_parameter("x", x_shape, F32, isOutput=False).ap()
    w_ap = nc.declare_dram_parameter("weight", [k, n], F32, isOutput=False).ap()
    b_ap = nc.declare_dram_parameter("bias", [P, n], F32, isOutput=False).ap()
    out_ap = nc.declare_dram_parameter("out", [m_shard, n], F32, isOutput=True).ap()
    # weight rows k = kt*P + p -> [p, kt, n]
    w_t = w_ap.rearrange("(kt p) n -> p kt n", p=P)

    with tile.TileContext(nc) as tc:
        with (
            tc.tile_pool(name="const", bufs=1) as const_pool,
            tc.tile_pool(name="xbt", bufs=1) as xbt_pool,
            tc.tile_pool(name="xstage", bufs=4) as xstage_pool,
            tc.tile_pool(name="xrh", bufs=6) as xrh_pool,
            tc.tile_pool(name="xbin", bufs=2) as xbin_pool,
            tc.tile_pool(name="wchunk", bufs=wbufs) as w_pool,
            tc.tile_pool(name="osb", bufs=4) as o_pool,
            tc.tile_pool(name="mm_psum", bufs=6, space="PSUM") as mm_psum,
            tc.tile_pool(name="tp_psum", bufs=2, space="PSUM") as tp_psum,
        ):
            ident = const_pool.tile([P, P], BF16)
            make_identity(nc, ident[:])
            bias_sb = const_pool.tile([P, n], F32)
            nc.sync.dma_start(bias_sb[:], b_ap[:, :])
            # per-partition tiny negative bias for the sign-binarize
            sgn_bias = const_pool.tile([P, 1], F32)
            nc.gpsimd.memset(sgn_bias[:], -1e-30)

            loop_cm = (
                tc.For_i(0, loop_k, 1) if loop_k > 1 else contextlib.nullcontext()
            )
            args = (
                nc, tc, x_ap, w_t, out_ap, bias_sb, sgn_bias, ident,
                xbt_pool, xstage_pool, xrh_pool, xbin_pool, w_pool, o_pool,
                mm_psum, tp_psum,
                m_shard, k, n, n_chunk, mt_n, kt_n, nt_n, hilo,
                xpose or XPOSE, wload or WLOAD,
            )
            if loop_scope == "b":
                # phase A once; the timing loop repeats only phase B —
                # measures matmul-stream steady state without the WAR
                # serialization of re-binarizing xbt each iteration
                xbt = _body(*args, phase="a")
                with loop_cm:
                    _body(*args, phase="b", xbt=xbt)
            else:
                with loop_cm:
                    _body(*args)

    nc.compile()
    return nc


def _body(
    nc, tc, x_ap, w_t, out_ap, bias_sb, sgn_bias, ident,
    xbt_pool, xstage_pool, xrh_pool, xbin_pool, w_pool, o_pool, mm_psum, tp_psum,
    m_shard, k, n, n_chunk, mt_n, kt_n, nt_n, hilo, xpose, wload,
    phase="ab", xbt=None,
):
    # Binarized-transposed activations, resident: [P(k), kt, m]
    if xbt is None:
        xbt = xbt_pool.tile([P, kt_n, m_shard], BF16)
    if "a" in phase and xpose == "host":
        # x is already [k, m_shard]; binarize straight into xbt, m-blocks
        # first so the first out-tiles' operands are ready early
        xt_t = x_ap.rearrange("(kt p) m -> p kt m", p=P)
        MB = min(m_shard, 256)
        for mb in range(m_shard // MB):
            m_bl = slice(mb * MB, (mb + 1) * MB)
            for kt in range(kt_n):
                xr = xrh_pool.tile([P, MB], F32)
                nc.sync.dma_start(xr[:], xt_t[:, kt, m_bl])
                # sign(x - tiny): exact zeros -> -1, matching where(x>0,1,-1)
                nc.scalar.sign(xbt[:, kt, m_bl], xr[:], bias=sgn_bias[:])
    elif "a" in phase:
        XH = min(k, 2048)
        for mt in range(mt_n):
            xb = xbin_pool.tile([P, k], BF16)
            for h in range(k // XH):
                xr = xstage_pool.tile([P, XH], F32)
                nc.sync.dma_start(
                    xr[:], x_ap[mt * P : (mt + 1) * P, h * XH : (h + 1) * XH]
                )
                # sign(x - tiny): exact zeros -> -1, matching where(x>0,1,-1)
                nc.scalar.sign(
                    xb[:, h * XH : (h + 1) * XH], xr[:], bias=sgn_bias[:]
                )
            for kt in range(kt_n):
                if xpose == "dma":
                    nc.sync.dma_start(
                        xbt[:, kt, mt * P : (mt + 1) * P],
                        xb[:, kt * P : (kt + 1) * P],
                        transpose=True,
                    )
                else:
                    tp = tp_psum.tile([P, P], BF16)
                    nc.tensor.transpose(
                        tp[:], xb[:, kt * P : (kt + 1) * P], ident[:]
                    )
                    nc.vector.tensor_copy(
                        xbt[:, kt, mt * P : (mt + 1) * P], tp[:]
                    )

    if "b" not in phase:
        return xbt
    for nt in range(nt_n):
        n_sl = slice(nt * n_chunk, (nt + 1) * n_chunk)
        wck = w_pool.tile([P, kt_n, n_chunk], BF16)
        if hilo:
            wlo = w_pool.tile([P, kt_n, n_chunk], BF16, tag="wlo")
        for kt in range(kt_n):
            if hilo:
                w32 = xstage_pool.tile([P, n_chunk], F32, tag="w32")
                nc.sync.dma_start(w32[:], w_t[:, kt, n_sl])
                nc.scalar.activation(
                    wck[:, kt, :], w32[:], mybir.ActivationFunctionType.Copy
                )
                nc.vector.tensor_tensor(
                    wlo[:, kt, :],
                    w32[:],
                    wck[:, kt, :],
                    mybir.AluOpType.subtract,
                )
            elif wload in ("act", "dve"):
                w32 = xstage_pool.tile([P, n_chunk], F32, tag="w32")
                nc.sync.dma_start(w32[:], w_t[:, kt, n_sl])
                if wload == "act":
                    nc.scalar.activation(
                        wck[:, kt, :], w32[:], mybir.ActivationFunctionType.Copy
                    )
                else:
                    nc.vector.tensor_copy(wck[:, kt, :], w32[:])
            else:
                # SWDGE cast-DMA: f32 DRAM -> bf16 SBUF
                nc.gpsimd.dma_start(wck[:, kt, :], w_t[:, kt, n_sl])
        for mt in range(mt_n):
            m_sl = slice(mt * P, (mt + 1) * P)
            ps = mm_psum.tile([P, n_chunk], F32)
            for kt in range(kt_n):
                nc.tensor.matmul(
                    ps[:],
                    xbt[:, kt, m_sl],
                    wck[:, kt, :],
                    start=(kt == 0),
                    stop=(kt == kt_n - 1) and not hilo,
                )
            if hilo:
                for kt in range(kt_n):
                    nc.tensor.matmul(
                        ps[:],
                        xbt[:, kt, m_sl],
                        wlo[:, kt, :],
                        start=False,
                        stop=(kt == kt_n - 1),
                    )
            osb = o_pool.tile([P, n_chunk], F32)
            nc.vector.tensor_add(osb[:], ps[:], bias_sb[:, n_sl])
            nc.sync.dma_start(out_ap[m_sl, n_sl], osb[:])


_NC_CACHE = {}


def _get_nc(cfg):
    nc = _NC_CACHE.get(cfg)
    if nc is None:
        nc = _NC_CACHE[cfg] = build_nc(*cfg)
    return nc


def kernel(x, weight, bias, _trace=False):
    x = np.ascontiguousarray(np.asarray(x, dtype=np.float32))
    weight = np.ascontiguousarray(np.asarray(weight, dtype=np.float32))
    bias = np.ascontiguousarray(np.asarray(bias, dtype=np.float32))
    tokens, k = x.shape
    n = weight.shape[1]
    m_shard = tokens // N_CORES
    assert tokens % N_CORES == 0

    bias_b = np.ascontiguousarray(np.broadcast_to(bias[None, :], (P, n)))
    if XPOSE == "host":
        xt = np.ascontiguousarray(x.T)  # [k, tokens]
        x_shards = [
            np.ascontiguousarray(xt[:, c * m_shard : (c + 1) * m_shard])
            for c in range(N_CORES)
        ]
    else:
        x_shards = [x[c * m_shard : (c + 1) * m_shard] for c in range(N_CORES)]
    in_maps = [
        {"x": x_shards[c], "weight": weight, "bias": bias_b}
        for c in range(N_CORES)
    ]
    nc = _get_nc((m_shard, k, n, 512, HILO, 1, XPOSE, WLOAD))
    res = run_bass_kernel_spmd(nc, in_maps, list(range(N_CORES)), trace=_trace)
    out = np.concatenate([res.results[c]["out"] for c in range(N_CORES)], axis=0)
    if _trace:
        return out, res
    return out

